# revision 1
# baseline (speedup 1.0000x reference)
"""Trainium2 Bass kernel for nn_Decoder (LSTM decoder + attention + lm_head).

Sharding: data-parallel over batch (64 -> 8 cores x 8). Each core runs the
full pipeline for its batch shard locally; no collectives.

Per-core pipeline (one NEFF):
  A) XG = X @ W_ih.T for all steps (one big bf16 matmul, token-major)
  B) 63 sequential LSTM cell steps: gates = XG[t] + h @ W_hh.T (bf16 matmuls,
     f32 PSUM), pointwise on ACT/DVE, then PE-transpose h into feature-major
     stores (bf16 for matmul reuse, f32 for the attention path)
  C) Attention (f32): Q = W_in @ H.T; per batch element: scores via matmul
     with host-pretransposed encodings, masked exp via ACT bias, unnormalized
     ctx + denominator via matmuls, normalize with DVE reciprocal
  C2) Output projection (bf16) + tanh
  D) Vocab projection (bf16): logits = OUT @ W_lm.T + b_lm, streamed over
     32000 vocab in 512-wide banks, bias added during PSUM eviction
"""
import sys

sys.path.insert(0, "/opt/trn_rl_repo")

import numpy as np
import ml_dtypes

from concourse import bacc, bass, mybir
from concourse.tile import TileContext
from concourse.bass_utils import run_bass_kernel_spmd

f32 = mybir.dt.float32
bf16 = mybir.dt.bfloat16
Act = mybir.ActivationFunctionType

NCORES = 8
T = 63            # decode steps (tgt_len - 1)
BL = 8            # batch per core
TOK = T * BL      # 504 tokens per core
TOKP = 512        # padded
SRC = 128
HID = 512
ENC = 512
INP = 512
V = 32000
GATES = 4 * HID   # 2048
NBANK = (V + 511) // 512  # 63 vocab banks (last = 256 wide)

# torch gate order i,f,g,o -> pipeline order f,i,g,o
PERM = np.concatenate([np.arange(512, 1024), np.arange(0, 512),
                       np.arange(1024, 1536), np.arange(1536, 2048)])

_BF = ml_dtypes.bfloat16


def _build(niter: int = 1, phases: str = "ABCD", dbg: bool = False,
           small_out: bool = False) -> "bacc.Bacc":
    nc = bacc.Bacc("TRN2", target_bir_lowering=False)

    xt_d = nc.dram_tensor("xt", [INP, TOKP], bf16, kind="ExternalInput")
    wih_d = nc.dram_tensor("wih", [INP, GATES], bf16, kind="ExternalInput")
    whh_d = nc.dram_tensor("whh", [HID, GATES], bf16, kind="ExternalInput")
    h0t_d = nc.dram_tensor("h0t", [128, 32], bf16, kind="ExternalInput")
    c0_d = nc.dram_tensor("c0", [BL, HID], f32, kind="ExternalInput")
    enc_d = nc.dram_tensor("encf", [BL * SRC, ENC], f32, kind="ExternalInput")
    enct_d = nc.dram_tensor("enctf", [BL * ENC, SRC], f32, kind="ExternalInput")
    mbt_d = nc.dram_tensor("mbt", [SRC, BL], f32, kind="ExternalInput")
    win_d = nc.dram_tensor("wint", [HID, ENC], f32, kind="ExternalInput")
    wout_d = nc.dram_tensor("woutt", [ENC + HID, HID], bf16, kind="ExternalInput")
    wlm_d = nc.dram_tensor("wlmt", [HID, V], bf16, kind="ExternalInput")
    bbc_d = nc.dram_tensor("bbc", [128, V], f32, kind="ExternalInput")
    id8b_d = nc.dram_tensor("id8b", [8, 8], bf16, kind="ExternalInput")
    id8f_d = nc.dram_tensor("id8f", [8, 8], f32, kind="ExternalInput")
    if small_out:
        out_d = nc.dram_tensor("logits", [128, 512], f32, kind="ExternalOutput")
    else:
        out_d = nc.dram_tensor("logits", [TOKP, V], f32, kind="ExternalOutput")

    with TileContext(nc) as tc:
        for i in range(niter):
            if i:
                tc.strict_bb_all_engine_barrier()
            _emit_iter(nc, tc, xt_d, wih_d, whh_d, h0t_d, c0_d, enc_d, enct_d,
                       mbt_d, win_d, wout_d, wlm_d, bbc_d, id8b_d, id8f_d, out_d,
                       phases=phases, dbg=dbg, small_out=small_out)
    nc.compile()
    return nc


def _emit_iter(nc, tc, xt_d, wih_d, whh_d, h0t_d, c0_d, enc_d, enct_d, mbt_d,
               win_d, wout_d, wlm_d, bbc_d, id8b_d, id8f_d, out_d,
               phases: str = "ABCD", dbg: bool = False, small_out: bool = False):
    MM = nc.tensor.matmul

    def dump(dst_row, tiles, width=TOKP):
        # debug: copy tiles (any dtype) as f32 into logits[dst_row:+128, i*width..]
        with tc.tile_pool(name="dbg", bufs=2) as dp:
            for i, tl in enumerate(tiles):
                s = dp.tile([128, width], f32, tag="d", name="dbgt")
                nc.vector.tensor_copy(s[:, 0:width], tl[:, 0:width])
                nc.sync.dma_start(
                    out=out_d[dst_row:dst_row + 128, i * width:(i + 1) * width],
                    in_=s[:, 0:width])

    with (
        tc.tile_pool(name="const", bufs=1) as cp,
        tc.tile_pool(name="store", bufs=1) as stp,
    ):
        # ---- resident tiles ----
        whh = [cp.tile([128, GATES], bf16, tag=f"whh{k}", name=f"whh{k}") for k in range(4)]
        for k in range(4):
            nc.sync.dma_start(out=whh[k][:], in_=whh_d[k * 128:(k + 1) * 128, :])
        h0t = cp.tile([128, 32], bf16, tag="h0t", name="h0t")
        nc.sync.dma_start(out=h0t[:], in_=h0t_d[:])
        c0 = cp.tile([BL, HID], f32, tag="c0", name="c0")
        nc.sync.dma_start(out=c0[:], in_=c0_d[:])
        enc_sb = [cp.tile([SRC, ENC], f32, tag=f"enc{b}", name=f"enc{b}") for b in range(BL)]
        for b in range(BL):
            nc.sync.dma_start(out=enc_sb[b][:], in_=enc_d[b * SRC:(b + 1) * SRC, :])
        enct_sb = [[cp.tile([128, SRC], f32, tag=f"enct{b}_{k}", name=f"enct{b}_{k}") for k in range(4)]
                   for b in range(BL)]
        for b in range(BL):
            for k in range(4):
                nc.sync.dma_start(out=enct_sb[b][k][:],
                                  in_=enct_d[b * ENC + k * 128: b * ENC + (k + 1) * 128, :])
        mbt = cp.tile([SRC, BL], f32, tag="mbt", name="mbt")
        nc.sync.dma_start(out=mbt[:], in_=mbt_d[:])
        wint = [cp.tile([128, ENC], f32, tag=f"wint{k}", name=f"wint{k}") for k in range(4)]
        for k in range(4):
            nc.sync.dma_start(out=wint[k][:], in_=win_d[k * 128:(k + 1) * 128, :])
        woutt = [cp.tile([128, HID], bf16, tag=f"woutt{k}", name=f"woutt{k}") for k in range(8)]
        for k in range(8):
            nc.sync.dma_start(out=woutt[k][:], in_=wout_d[k * 128:(k + 1) * 128, :])
        id8b = cp.tile([8, 8], bf16, tag="id8b", name="id8b")
        nc.sync.dma_start(out=id8b[:], in_=id8b_d[:])
        id8f = cp.tile([8, 8], f32, tag="id8f", name="id8f")
        nc.sync.dma_start(out=id8f[:], in_=id8f_d[:])
        ones_s = cp.tile([128, 1], f32, tag="ones_s", name="ones_s")
        nc.any.memset(ones_s[:], 1.0)
        ones_1 = cp.tile([1, 128], f32, tag="ones_1", name="ones_1")
        nc.any.memset(ones_1[:], 1.0)

        # ---- accumulating stores ----
        xg = [stp.tile([128, GATES], bf16, tag=f"xg{m}", name=f"xg{m}") for m in range(4)]
        htbB = stp.tile([128, 4 * TOKP], bf16, tag="htbB", name="htbB")
        htfB = stp.tile([128, 4 * TOKP], f32, tag="htfB", name="htfB")
        htb = [htbB[:, k * TOKP:(k + 1) * TOKP] for k in range(4)]
        htf = [htfB[:, k * TOKP:(k + 1) * TOKP] for k in range(4)]
        qtf = [stp.tile([128, TOKP], f32, tag=f"qtf{m}", name=f"qtf{m}") for m in range(4)]
        htfbm = [stp.tile([128, TOKP], f32, tag=f"htfbm{k}", name=f"htfbm{k}") for k in range(4)]
        ctxt = [stp.tile([128, TOKP], bf16, tag=f"ctxt{k}", name=f"ctxt{k}") for k in range(4)]
        outt = [stp.tile([128, TOKP], bf16, tag=f"outt{m}", name=f"outt{m}") for m in range(4)]

        # ================= Phase A: XG = X @ W_ih.T =================
        if "A" not in phases:
            return
        with (
            tc.tile_pool(name="pa_sb", bufs=1) as pa_sb,
            tc.tile_pool(name="pa_ps", bufs=2, space="PSUM") as pa_ps,
        ):
            xts = [pa_sb.tile([128, TOKP], bf16, tag=f"xt{k}", name=f"xt{k}") for k in range(4)]
            for k in range(4):
                nc.sync.dma_start(out=xts[k][:], in_=xt_d[k * 128:(k + 1) * 128, :])
            wih = [pa_sb.tile([128, GATES], bf16, tag=f"wih{k}", name=f"wih{k}") for k in range(4)]
            for k in range(4):
                nc.sync.dma_start(out=wih[k][:], in_=wih_d[k * 128:(k + 1) * 128, :])
            for m in range(4):
                for nb in range(4):
                    ps = pa_ps.tile([128, 512], f32, tag="pa", name="pa")
                    for k in range(4):
                        MM(ps[:], xts[k][:, m * 128:(m + 1) * 128],
                           wih[k][:, nb * 512:(nb + 1) * 512],
                           start=(k == 0), stop=(k == 3))
                    nc.scalar.copy(xg[m][:, nb * 512:(nb + 1) * 512], ps[:])

        if dbg:
            dump(0, xg, width=GATES)
        # ================= Phase B: LSTM recurrence =================
        if "B" not in phases:
            return
        with (
            tc.tile_pool(name="pb_stage", bufs=3) as pb_stage,
            tc.tile_pool(name="pb_tmp", bufs=2) as pb_tmp,
            tc.tile_pool(name="pb_c", bufs=2) as pb_c,
            tc.tile_pool(name="pb_g", bufs=1, space="PSUM") as pb_g,
            tc.tile_pool(name="pb_tr", bufs=2, space="PSUM") as pb_tr,
        ):
            c_prev = c0
            for t in range(T):
                hT = h0t if t == 0 else None  # feature-major h_t source
                stg = pb_stage.tile([BL, GATES], bf16, tag="stg", name="stg")
                m, r = t // 16, t % 16
                nc.sync.dma_start(out=stg[:], in_=xg[m][r * 8:(r + 1) * 8, :])
                ps = pb_g.tile([BL, GATES], f32, tag="gates", name="gates")
                for nb in range(4):
                    bank = ps[:, nb * 512:(nb + 1) * 512]
                    MM(bank, id8b[:], stg[:, nb * 512:(nb + 1) * 512],
                       start=True, stop=False)
                    for k in range(4):
                        lhs = (hT[:, k * 8:(k + 1) * 8] if hT is not None
                               else htb[k][:, (t - 1) * 8:t * 8])
                        MM(bank, lhs, whh[k][:, nb * 512:(nb + 1) * 512],
                           start=False, stop=(k == 3))
                # pointwise: banks 0=f 1=i 2=g 3=o. All ACT ops are Sigmoid
                # (tanh(x) = 2*sigmoid(2x)-1, affine part folded into DVE ops)
                # to avoid 1.28us ACT table swaps between Sigmoid and Tanh.
                sigf = pb_tmp.tile([BL, HID], f32, tag="sigf", name="sigf")
                nc.scalar.activation(sigf[:], ps[:, 0:512], Act.Sigmoid)
                c1 = pb_tmp.tile([BL, HID], f32, tag="c1", name="c1")
                nc.vector.tensor_mul(c1[:], sigf[:], c_prev[:])
                sigi = pb_tmp.tile([BL, HID], f32, tag="sigi", name="sigi")
                nc.scalar.activation(sigi[:], ps[:, 512:1024], Act.Sigmoid)
                tgs = pb_tmp.tile([BL, HID], f32, tag="tgs", name="tgs")
                nc.scalar.activation(tgs[:], ps[:, 1024:1536], Act.Sigmoid, scale=2.0)
                # u' = (tgs - 0.5) * sigi  == sigi*tanh(g)/2
                up = pb_tmp.tile([BL, HID], f32, tag="up", name="up")
                nc.vector.scalar_tensor_tensor(
                    up[:], tgs[:], 0.5, sigi[:],
                    op0=mybir.AluOpType.subtract, op1=mybir.AluOpType.mult)
                # c = c1 + 2*u'
                c_new = pb_c.tile([BL, HID], f32, tag="c", name="c")
                nc.vector.scalar_tensor_tensor(
                    c_new[:], up[:], 2.0, c1[:],
                    op0=mybir.AluOpType.mult, op1=mybir.AluOpType.add)
                sigo = pb_tmp.tile([BL, HID], f32, tag="sigo", name="sigo")
                nc.scalar.activation(sigo[:], ps[:, 1536:2048], Act.Sigmoid)
                tcs = pb_tmp.tile([BL, HID], f32, tag="tcs", name="tcs")
                nc.scalar.activation(tcs[:], c_new[:], Act.Sigmoid, scale=2.0)
                # h' = (tcs - 0.5) * sigo == h/2 ; the x2 is folded into the
                # post-transpose store copies below.
                h = pb_tmp.tile([BL, HID], f32, tag="h", name="h")
                nc.vector.scalar_tensor_tensor(
                    h[:], tcs[:], 0.5, sigo[:],
                    op0=mybir.AluOpType.subtract, op1=mybir.AluOpType.mult)
                # transpose h' (8,512) -> feature-major stores (one PSUM tile,
                # then a single strided copy into each big store, scaled x2)
                pst = pb_tr.tile([128, 32], f32, tag="tr", name="tr")
                for k in range(4):
                    nc.tensor.transpose(pst[:, k * 8:(k + 1) * 8],
                                        h[:, k * 128:(k + 1) * 128], id8f[:])
                pstv = pst[:].rearrange("p (k c) -> p k c", k=4)
                htbv = htbB[:].rearrange("p (k c) -> p k c", k=4)[:, :, t * 8:(t + 1) * 8]
                htfv = htfB[:].rearrange("p (k c) -> p k c", k=4)[:, :, t * 8:(t + 1) * 8]
                nc.scalar.mul(htbv, pstv, 2.0)
                nc.vector.tensor_scalar_mul(htfv, pstv, 2.0)
                c_prev = c_new

        if dbg:
            dump(128, htf, width=TOK)
        # ================= Phase C: attention (f32) =================
        if "C" not in phases:
            return
        with tc.tile_pool(name="pq_ps", bufs=2, space="PSUM") as pq_ps:
            # reorder H columns token-major -> b-major once, so every matmul
            # in the attention phase streams contiguous operands
            for k in range(4):
                hv = htf[k][:, 0:TOK].rearrange("p (j b) -> p b j", b=BL)
                bv = htfbm[k][:, 0:TOK].rearrange("p (b j) -> p b j", b=BL)
                nc.vector.tensor_copy(bv, hv)
            for m in range(4):
                ps = pq_ps.tile([128, TOK], f32, tag="q", name="q")
                for k in range(4):
                    MM(ps[:], wint[k][:, m * 128:(m + 1) * 128],
                       htfbm[k][:, 0:TOK], start=(k == 0), stop=(k == 3))
                nc.scalar.copy(qtf[m][:, 0:TOK], ps[:])

        TB = T * BL  # 504, b-major stage layout: col = b*T + j
        with (
            tc.tile_pool(name="pc_sb", bufs=1) as pc_sb,
            tc.tile_pool(name="pc_s", bufs=1, space="PSUM") as pc_s,
            tc.tile_pool(name="pc_d", bufs=1, space="PSUM") as pc_d,
            tc.tile_pool(name="pc_b", bufs=1, space="PSUM") as pc_b,
            tc.tile_pool(name="pc_c", bufs=1, space="PSUM") as pc_c,
        ):
            pss = pc_s.tile([SRC, TB], f32, tag="scores", name="scores")
            for b in range(BL):
                for k in range(4):
                    MM(pss[:, b * T:(b + 1) * T], enct_sb[b][k][:],
                       qtf[k][:, b * T:(b + 1) * T], start=(k == 0), stop=(k == 3))
            e_all = pc_sb.tile([SRC, TB], f32, tag="e_all", name="e_all")
            for b in range(BL):
                nc.scalar.activation(e_all[:, b * T:(b + 1) * T],
                                     pss[:, b * T:(b + 1) * T], Act.Exp,
                                     bias=mbt[:, b:b + 1])
            psd = pc_d.tile([1, TB], f32, tag="denom", name="denom")
            MM(psd[:], ones_s[:], e_all[:], start=True, stop=True)
            rec = pc_sb.tile([1, TB], f32, tag="rec", name="rec")
            nc.vector.reciprocal(rec[:], psd[:])
            psb = pc_b.tile([128, TB], f32, tag="recb_ps", name="recb_ps")
            MM(psb[:], ones_1[:], rec[:], start=True, stop=True)
            recb = pc_sb.tile([128, TB], f32, tag="recb", name="recb")
            nc.scalar.copy(recb[:], psb[:])
            recv = recb[:].rearrange("p (b j) -> p b j", b=BL)
            for k in range(4):
                psc = pc_c.tile([128, TB], f32, tag=f"ctx{k}", name=f"ctx{k}")
                for b in range(BL):
                    MM(psc[:, b * T:(b + 1) * T],
                       enc_sb[b][:, k * 128:(k + 1) * 128],
                       e_all[:, b * T:(b + 1) * T], start=True, stop=True)
                # normalize + scatter b-major -> token-major in one strided mul
                ctxv = ctxt[k][:, 0:TOK].rearrange("p (j b) -> p b j", b=BL)
                pscv = psc[:].rearrange("p (b j) -> p b j", b=BL)
                nc.vector.tensor_mul(ctxv, pscv, recv)

        if dbg:
            dump(256, qtf, width=TOK)
            dump(384, ctxt, width=TOK)
        # ================= Phase C2: out-projection + tanh =================
        with tc.tile_pool(name="po_ps", bufs=2, space="PSUM") as po_ps:
            for m in range(4):
                ps = po_ps.tile([128, TOK], f32, tag="o", name="o")
                for k in range(8):
                    rhs = ctxt[k] if k < 4 else htb[k - 4]
                    MM(ps[:], woutt[k][:, m * 128:(m + 1) * 128],
                       rhs[:, 0:TOK], start=(k == 0), stop=(k == 7))
                nc.scalar.activation(outt[m][:, 0:TOK], ps[:], Act.Tanh)

        if dbg:
            dump(0, outt, width=TOK)
        # ================= Phase D: vocab projection =================
        if "D" not in phases:
            return
        with (
            tc.tile_pool(name="pd_w", bufs=2) as pd_w,
            tc.tile_pool(name="pd_b", bufs=2) as pd_b,
            tc.tile_pool(name="pd_st", bufs=4) as pd_st,
            tc.tile_pool(name="pd_ps", bufs=4, space="PSUM") as pd_ps,
        ):
            for nb in range(NBANK):
                n0 = nb * 512
                nw = min(512, V - n0)
                wl = [pd_w.tile([128, 512], bf16, tag=f"wl{k}", name=f"wl{k}") for k in range(4)]
                for k in range(4):
                    nc.sync.dma_start(out=wl[k][:, 0:nw],
                                      in_=wlm_d[k * 128:(k + 1) * 128, n0:n0 + nw])
                bb = pd_b.tile([128, 512], f32, tag="bb", name="bb")
                nc.sync.dma_start(out=bb[:, 0:nw], in_=bbc_d[:, n0:n0 + nw])
                for mt in range(4):
                    m0 = mt * 128
                    mw = min(128, TOK - m0)
                    ps = pd_ps.tile([128, 512], f32, tag="v", name="v")
                    for k in range(4):
                        MM(ps[0:mw, 0:nw], outt[k][:, m0:m0 + mw], wl[k][:, 0:nw],
                           start=(k == 0), stop=(k == 3))
                    st = pd_st.tile([128, 512], f32, tag="st", name="st")
                    nc.vector.tensor_add(st[0:mw, 0:nw], ps[0:mw, 0:nw],
                                         bb[0:mw, 0:nw])
                    dst = (out_d[0:mw, 0:nw] if small_out
                           else out_d[m0:m0 + mw, n0:n0 + nw])
                    nc.sync.dma_start(out=dst, in_=st[0:mw, 0:nw])


def _prep_in_maps(inputs: dict) -> list[dict]:
    targets = np.asarray(inputs["targets"])
    mask = np.asarray(inputs["attention_mask"])
    enc = np.asarray(inputs["encodings"], dtype=np.float32)
    h = np.asarray(inputs["h"], dtype=np.float32)
    c = np.asarray(inputs["c"], dtype=np.float32)
    emb = np.asarray(inputs["emb"], dtype=np.float32)
    W_ih = np.asarray(inputs["W_ih"], dtype=np.float32)
    W_hh = np.asarray(inputs["W_hh"], dtype=np.float32)
    W_in = np.asarray(inputs["W_in"], dtype=np.float32)
    W_out = np.asarray(inputs["W_out"], dtype=np.float32)
    W_lm = np.asarray(inputs["W_lm"], dtype=np.float32)
    b_lm = np.asarray(inputs["b_lm"], dtype=np.float32)

    x_seq = emb[targets[:-1]]                      # (63, 64, 512)
    wih_p = W_ih[PERM].T.astype(_BF).copy()        # (512, 2048)
    whh_p = W_hh[PERM].T.astype(_BF).copy()
    wint = W_in.T.astype(np.float32).copy()        # (512, 512)
    woutt = W_out.T.astype(_BF).copy()             # (1024, 512)
    wlmt = W_lm.T.astype(_BF).copy()               # (512, 32000)
    bbc = np.broadcast_to(b_lm, (128, V)).astype(np.float32).copy()
    id8b = np.eye(8, dtype=_BF)
    id8f = np.eye(8, dtype=np.float32)

    in_maps = []
    for cidx in range(NCORES):
        sl = slice(cidx * BL, (cidx + 1) * BL)
        xt = np.zeros((INP, TOKP), np.float32)
        xt[:, :TOK] = x_seq[:, sl, :].reshape(TOK, INP).T
        h0t = np.ascontiguousarray(h[sl].T).reshape(4, 128, BL)   # (k, 128, 8)
        h0t = np.concatenate([h0t[k] for k in range(4)], axis=1)  # (128, 32)
        encc = enc[:, sl, :]                                      # (128, 8, 512)
        encf = np.ascontiguousarray(encc.transpose(1, 0, 2)).reshape(BL * SRC, ENC)
        enctf = np.ascontiguousarray(encc.transpose(1, 2, 0)).reshape(BL * ENC, SRC)
        mbt = np.where(mask[:, sl], np.float32(-1e30), np.float32(0.0)).astype(np.float32)
        in_maps.append({
            "xt": xt.astype(_BF),
            "wih": wih_p, "whh": whh_p,
            "h0t": h0t.astype(_BF),
            "c0": c[sl].astype(np.float32),
            "encf": encf.astype(np.float32),
            "enctf": enctf.astype(np.float32),
            "mbt": mbt,
            "wint": wint, "woutt": woutt, "wlmt": wlmt, "bbc": bbc,
            "id8b": id8b, "id8f": id8f,
        })
    return in_maps


def _assemble(results) -> np.ndarray:
    out = np.empty((T, 64, V), np.float32)
    for cidx in range(NCORES):
        lg = results[cidx]["logits"][:TOK].reshape(T, BL, V)
        out[:, cidx * BL:(cidx + 1) * BL, :] = lg
    return out


_CACHE: dict = {}


def kernel(**inputs) -> np.ndarray:
    if "nc" not in _CACHE:
        _CACHE["nc"] = _build(niter=1)
    in_maps = _prep_in_maps(inputs)
    res = run_bass_kernel_spmd(_CACHE["nc"], in_maps, core_ids=list(range(NCORES)))
    return _assemble(res.results)

